# revision 3
# baseline (speedup 1.0000x reference)
"""SwiGLU FFN (gate/up/down) on 8 TRN2 NeuronCores.

Strategy: data-parallel over tokens. Each core gets 1024 tokens and the
full weight set. All matmuls run in bf16 with fp32 PSUM accumulation.

Layout trick: activations are kept transposed on-chip (feature dim on
partitions, tokens on the free dim), so every matmul has its contraction
dim on partitions for both operands and no on-device transposes are
needed:
  gate.T = Wg_lhsT.T @ x.T    (lhsT[k,m] = Wg[m,k], k = hidden)
  h.T    = silu(gate.T) * up.T
  y.T    = Wd_lhsT.T @ h.T    (lhsT[k,m] = Wd[m,k], k = inter)

Weights are pre-tiled on the host into [m_tile, p, (g), k_tile, m] order
so each per-m-tile DMA reads 16-22KB contiguous per partition.

SBUF budget per partition (of ~208KB usable): xT 32KB + hT 86KB +
weight slots 2x22KB + staging ~12KB.
"""

import numpy as np
import ml_dtypes

import concourse.bass as bass
import concourse.tile as tile
import concourse.mybir as mybir
from concourse.bass_utils import run_bass_kernel_spmd

BF16 = ml_dtypes.bfloat16

P = 128
HID = 4096
INT = 11008
TOK = 8192
NCORES = 8
TPC = TOK // NCORES          # tokens per core
T = 512                      # tokens per pass (PSUM free-dim limit, f32)
NPASS = TPC // T
KTH = HID // P               # 32 k-tiles over hidden
MTI = INT // P               # 86 m-tiles over intermediate
MTH = HID // P               # 32 m-tiles over hidden (down proj)
KTI = INT // P               # 86 k-tiles over intermediate


def _thin_mm_sem_updates(nc):
    """Drop per-matmul semaphore updates except on stop=1 (group-final) MMs,
    remapping all waits on those semaphores to the coarser counter values.
    Consumers only ever wait at accumulation-group boundaries, so group-final
    increments carry the same information at 1/32 the EVT_SEM write cost
    (~26 ns of PE engine time per increment).

    Two-phase: plan (validate every assumption, no mutation) then apply.
    If the emitted BIR doesn't match the expected shape, skip thinning —
    the kernel stays correct, just with the per-MM increments."""
    try:
        fn = nc.m.functions[0]
        sem_ups = {}
        for b in fn.blocks:
            for inst in b.instructions:
                if type(inst).__name__ != "InstMatmult":
                    continue
                si = inst.sync_info
                if not si or not si.on_update:
                    continue
                assert len(si.on_update) == 1, inst.name
                u = si.on_update[0]
                assert u.update_value == 1, (inst.name, u.update_value)
                key = u.ant_name or u.id
                sem_ups.setdefault(key, []).append(
                    (inst, bool(inst.stop_tensor_calc))
                )

        remap = {}
        for key, ups in sem_ups.items():
            n = len(ups)
            keeps_prefix = []
            cnt = 0
            for (_, keep) in ups:
                if keep:
                    cnt += 1
                keeps_prefix.append(cnt)
            next_keep = [None] * n
            cur = None
            for i in range(n - 1, -1, -1):
                if ups[i][1]:
                    cur = keeps_prefix[i]
                next_keep[i] = cur
            remap[key] = (next_keep, n)

        plan = []  # (inst, new_sync_info) applied only if the whole plan builds
        for b in fn.blocks:
            for inst in b.instructions:
                si = getattr(inst, "sync_info", None)
                if not si or not si.on_wait:
                    continue
                new_waits = []
                changed = False
                for w in si.on_wait:
                    key = w.ant_name or w.id
                    if key in remap and w.wait_value is not None and w.wait_value > 0:
                        assert w.wait_mode == "sem-ge-imm", (inst.name, w.wait_mode)
                        next_keep, n = remap[key]
                        assert w.wait_value <= n, (inst.name, w.wait_value, n)
                        nv = next_keep[w.wait_value - 1]
                        assert nv is not None, \
                            f"wait beyond last kept update {inst.name}"
                        new_waits.append(mybir.SyncWait(
                            sync_type=w.sync_type, id=w.id, ant_name=w.ant_name,
                            wait_mode=w.wait_mode, wait_value=nv,
                        ))
                        changed = True
                    else:
                        new_waits.append(w)
                if changed:
                    plan.append((inst, mybir.SyncInfo(
                        on_wait=new_waits, on_update=list(si.on_update))))
        drops = [inst for ups in sem_ups.values()
                 for (inst, keep) in ups if not keep]
    except AssertionError:
        return nc

    for inst, si in plan:
        inst.sync_info = si
    for inst in drops:  # after remaps so a remapped wait on the same MM survives
        inst.sync_info = mybir.SyncInfo(
            on_wait=list(inst.sync_info.on_wait), on_update=[]
        )
    return nc


def _split_multiwaits(nc):
    # This walrus build supports a single sync-wait slot per instruction;
    # hoist extra waits onto single-wait NoOps inserted just before the
    # offending instruction on the same engine (same semantics: the engine
    # stream blocks on each wait in order).
    n = 0
    for f in nc.m.functions:
        for blk in f.blocks:
            insts = blk.instructions  # live list
            i = 0
            while i < len(insts):
                inst = insts[i]
                si = getattr(inst, "sync_info", None)
                if si is not None and si.on_wait and len(si.on_wait) > 1:
                    waits = list(si.on_wait)
                    for j, w in enumerate(waits[:-1]):
                        nop = mybir.InstNoOp(
                            name=f"{inst.name}_splitwait{j}", ins=[], outs=[]
                        )
                        nop.engine = inst.engine
                        nop.sync_info = mybir.SyncInfo(on_wait=[w], on_update=[])
                        insts.insert(i, nop)
                        i += 1
                        n += 1
                    inst.sync_info = mybir.SyncInfo(
                        on_wait=[waits[-1]], on_update=list(si.on_update)
                    )
                i += 1
    return n


def build_nc():
    bf = mybir.dt.bfloat16
    f32 = mybir.dt.float32
    nc = bass.Bass()

    xt = nc.dram_tensor("xt", [NPASS, P, KTH, T], bf, kind="ExternalInput")
    wgu = nc.dram_tensor("wgu", [MTI, P, 2, KTH, P], bf, kind="ExternalInput")
    wd = nc.dram_tensor("wd", [MTH, P, KTI, P], bf, kind="ExternalInput")
    yt = nc.dram_tensor("yt", [NPASS, MTH, P, T], f32, kind="ExternalOutput")

    with tile.TileContext(nc) as tc:
        with (
            tc.tile_pool(name="xp", bufs=1) as xp,
            tc.tile_pool(name="hp", bufs=1) as hp,
            tc.tile_pool(name="wp", bufs=2) as wp,
            tc.tile_pool(name="sp", bufs=3) as sp,
            tc.tile_pool(name="pg", bufs=3, space="PSUM") as pg,
            tc.tile_pool(name="py", bufs=2, space="PSUM") as py,
        ):
            for ps in range(NPASS):
                xt_sb = xp.tile([P, KTH, T], bf, name="xt_sb", tag="xt_sb")
                # xt on the Scalar engine's DMA queue (parallel with the
                # weight stream on qSync), chunked so the first matmuls
                # start as soon as the first k-tiles land.
                for c in range(4):
                    k0, k1 = c * (KTH // 4), (c + 1) * (KTH // 4)
                    nc.scalar.dma_start(xt_sb[:, k0:k1], xt[ps, :, k0:k1])
                ht = hp.tile([P, MTI, T], bf, name="ht", tag="ht")
                for mt in range(MTI):
                    w = wp.tile([P, 2, KTH, P], bf, name="w_gu", tag="w")
                    if ps == 0 and mt == 0:
                        # chunk the very first weight tile so MM(kt=0) can
                        # start before the whole 2MB tile arrives
                        for c in range(4):
                            k0, k1 = c * (KTH // 4), (c + 1) * (KTH // 4)
                            nc.sync.dma_start(
                                w[:, :, k0:k1], wgu[mt, :, :, k0:k1]
                            )
                    else:
                        nc.sync.dma_start(w[:], wgu[mt])
                    g_ps = pg.tile([P, T], f32, name="g_ps", tag="g")
                    u_ps = pg.tile([P, T], f32, name="u_ps", tag="u")
                    for kt in range(KTH):
                        nc.tensor.matmul(
                            g_ps[:], w[:, 0, kt], xt_sb[:, kt],
                            start=(kt == 0), stop=(kt == KTH - 1),
                        )
                    for kt in range(KTH):
                        nc.tensor.matmul(
                            u_ps[:], w[:, 1, kt], xt_sb[:, kt],
                            start=(kt == 0), stop=(kt == KTH - 1),
                        )
                    sil = sp.tile([P, T], f32, name="sil", tag="sil")
                    nc.scalar.activation(
                        sil[:], g_ps[:], mybir.ActivationFunctionType.Silu
                    )
                    nc.vector.tensor_mul(ht[:, mt], sil[:], u_ps[:])
                for mh in range(MTH):
                    wdt = wp.tile([P, KTI, P], bf, name="w_d", tag="w")
                    nc.sync.dma_start(wdt[:], wd[mh])
                    y_ps = py.tile([P, T], f32, name="y_ps", tag="y")
                    for kt in range(KTI):
                        nc.tensor.matmul(
                            y_ps[:], wdt[:, kt], ht[:, kt],
                            start=(kt == 0), stop=(kt == KTI - 1),
                        )
                    y_sb = sp.tile([P, T], f32, name="y_sb", tag="ysb")
                    nc.vector.tensor_copy(y_sb[:], y_ps[:])
                    nc.sync.dma_start(yt[ps, mh], y_sb[:])

    _thin_mm_sem_updates(nc)
    _split_multiwaits(nc)
    return nc


def prep_inputs(x, W_gate, W_up, W_down):
    # lhsT layouts: element [mt, p, (g,) kt, m] = W[mt*128+m, kt*128+p]
    wg = W_gate.reshape(MTI, P, KTH, P).transpose(0, 3, 2, 1)
    wu = W_up.reshape(MTI, P, KTH, P).transpose(0, 3, 2, 1)
    wgu = np.stack([wg, wu], axis=2).astype(BF16)          # [mt, p, 2, kt, m]
    wd = W_down.reshape(MTH, P, KTI, P).transpose(0, 3, 2, 1).astype(BF16)
    # x: [core, pass, t, kt, p] -> per-core [pass, p, kt, t]
    xr = x.reshape(NCORES, NPASS, T, KTH, P)
    xts = [np.ascontiguousarray(xr[c].transpose(0, 3, 2, 1)).astype(BF16)
           for c in range(NCORES)]
    return xts, wgu, wd


_NC_CACHE = []


def get_nc():
    if not _NC_CACHE:
        _NC_CACHE.append(build_nc())
    return _NC_CACHE[0]


def make_in_maps(x, W_gate, W_up, W_down):
    xts, wgu, wd = prep_inputs(
        np.asarray(x, np.float32),
        np.asarray(W_gate, np.float32),
        np.asarray(W_up, np.float32),
        np.asarray(W_down, np.float32),
    )
    return [{"xt": xts[c], "wgu": wgu, "wd": wd} for c in range(NCORES)]


def unshard_output(res_by_name):
    yt = np.asarray(res_by_name["yt"]).reshape(NCORES, NPASS, MTH, P, T)
    out = np.empty((TOK, HID), np.float32)
    for c in range(NCORES):
        out[c * TPC:(c + 1) * TPC] = (
            yt[c].transpose(0, 3, 1, 2).reshape(TPC, HID)
        )
    return out


def kernel(x, W_gate, W_up, W_down):
    nc = get_nc()
    in_maps = make_in_maps(x, W_gate, W_up, W_down)
    res = run_bass_kernel_spmd(nc, in_maps, core_ids=list(range(NCORES)))
    return unshard_output(
        {"yt": np.stack([res.results[c]["yt"] for c in range(NCORES)]).reshape(
            NCORES * NPASS, MTH, P, T
        )}
    )



# revision 7
# speedup vs baseline: 1.0768x; 1.0768x over previous
"""SwiGLU FFN (gate/up/down) on 8 TRN2 NeuronCores.

Strategy: data-parallel over tokens. Each core gets 1024 tokens and the
full weight set. All matmuls run in bf16 with fp32 PSUM accumulation.

Layout trick: activations are kept transposed on-chip (feature dim on
partitions, tokens on the free dim), so every matmul has its contraction
dim on partitions for both operands and no on-device transposes are
needed:
  gate.T = Wg_lhsT.T @ x.T    (lhsT[k,m] = Wg[m,k], k = hidden)
  h.T    = silu(gate.T) * up.T
  y.T    = Wd_lhsT.T @ h.T    (lhsT[k,m] = Wd[m,k], k = inter)

Weights are pre-tiled on the host into [m_tile, p, (g), k_tile, m] order
so each per-m-tile DMA reads 16-22KB contiguous per partition.

SBUF budget per partition (of ~208KB usable): xT 32KB + hT 86KB +
weight slots 2x22KB + staging ~12KB.
"""

import numpy as np
import ml_dtypes

import concourse.bass as bass
import concourse.tile as tile
import concourse.mybir as mybir
from concourse.bass_utils import run_bass_kernel_spmd

BF16 = ml_dtypes.bfloat16

P = 128
HID = 4096
INT = 11008
TOK = 8192
NCORES = 8
TPC = TOK // NCORES          # tokens per core
T = 512                      # tokens per pass (PSUM free-dim limit, f32)
NPASS = TPC // T
KTH = HID // P               # 32 k-tiles over hidden
MTI = INT // P               # 86 m-tiles over intermediate
MTH = HID // P               # 32 m-tiles over hidden (down proj)
KTI = INT // P               # 86 k-tiles over intermediate


def _thin_mm_sem_updates(nc):
    """Drop per-matmul semaphore updates except on stop=1 (group-final) MMs,
    remapping all waits on those semaphores to the coarser counter values.
    Consumers only ever wait at accumulation-group boundaries, so group-final
    increments carry the same information at 1/32 the EVT_SEM write cost
    (~26 ns of PE engine time per increment).

    Two-phase: plan (validate every assumption, no mutation) then apply.
    If the emitted BIR doesn't match the expected shape, skip thinning —
    the kernel stays correct, just with the per-MM increments."""
    try:
        fn = nc.m.functions[0]
        sem_ups = {}
        for b in fn.blocks:
            for inst in b.instructions:
                if type(inst).__name__ != "InstMatmult":
                    continue
                si = inst.sync_info
                if not si or not si.on_update:
                    continue
                assert len(si.on_update) == 1, inst.name
                u = si.on_update[0]
                assert u.update_value == 1, (inst.name, u.update_value)
                key = u.ant_name or u.id
                sem_ups.setdefault(key, []).append(
                    (inst, bool(inst.stop_tensor_calc))
                )

        remap = {}
        for key, ups in sem_ups.items():
            n = len(ups)
            keeps_prefix = []
            cnt = 0
            for (_, keep) in ups:
                if keep:
                    cnt += 1
                keeps_prefix.append(cnt)
            next_keep = [None] * n
            cur = None
            for i in range(n - 1, -1, -1):
                if ups[i][1]:
                    cur = keeps_prefix[i]
                next_keep[i] = cur
            remap[key] = (next_keep, n)

        plan = []  # (inst, new_sync_info) applied only if the whole plan builds
        for b in fn.blocks:
            for inst in b.instructions:
                si = getattr(inst, "sync_info", None)
                if not si or not si.on_wait:
                    continue
                new_waits = []
                changed = False
                for w in si.on_wait:
                    key = w.ant_name or w.id
                    if key in remap and w.wait_value is not None and w.wait_value > 0:
                        assert w.wait_mode == "sem-ge-imm", (inst.name, w.wait_mode)
                        next_keep, n = remap[key]
                        assert w.wait_value <= n, (inst.name, w.wait_value, n)
                        nv = next_keep[w.wait_value - 1]
                        assert nv is not None, \
                            f"wait beyond last kept update {inst.name}"
                        new_waits.append(mybir.SyncWait(
                            sync_type=w.sync_type, id=w.id, ant_name=w.ant_name,
                            wait_mode=w.wait_mode, wait_value=nv,
                        ))
                        changed = True
                    else:
                        new_waits.append(w)
                if changed:
                    plan.append((inst, mybir.SyncInfo(
                        on_wait=new_waits, on_update=list(si.on_update))))
        drops = [inst for ups in sem_ups.values()
                 for (inst, keep) in ups if not keep]
    except AssertionError:
        return nc

    for inst, si in plan:
        inst.sync_info = si
    for inst in drops:  # after remaps so a remapped wait on the same MM survives
        inst.sync_info = mybir.SyncInfo(
            on_wait=list(inst.sync_info.on_wait), on_update=[]
        )
    return nc


def _hoist_group_entry_waits(nc, lookback=4):
    """Move the sem waits off group-entry matmuls onto a NoOp inserted
    `lookback` PE-instructions earlier.

    A wait sitting between matmuls on the PE queue blocks the engine's
    LDWEIGHTS pull-ahead, costing ~216 ns (one full MM slot) at every
    accumulation-group boundary (~408 of them, ~80 us total). Processed a
    few MMs early, the wait is long-satisfied (its producers finished >=1
    group earlier: psum-free consumers and weight DMAs lag the PE stream by
    a full iteration) and the boundary MM issues back-to-back.

    Safe because hoisting a wait earlier on the same in-order engine stream
    only ever waits sooner, and none of the waited semaphores depend on the
    MMs being hoisted past (they complete >=1 full group before). Skips
    boundary MMs with fewer than `lookback` preceding PE instructions."""
    try:
        fn = nc.m.functions[0]
        for blk in fn.blocks:
            insts = blk.instructions  # live list
            pe_engine = None
            for ins in insts:
                if type(ins).__name__ == "InstMatmult":
                    pe_engine = ins.engine
                    break
            if pe_engine is None:
                continue
            i = 0
            n_before = 0  # PE insts seen so far; position of k-th-back PE inst
            pe_positions = []  # indices of PE-engine instructions
            # collect, then plan insertions back-to-front so indices stay valid
            for idx, ins in enumerate(insts):
                if getattr(ins, "engine", None) == pe_engine:
                    pe_positions.append(idx)
            plan = []
            for r, idx in enumerate(pe_positions):
                ins = insts[idx]
                if type(ins).__name__ != "InstMatmult":
                    continue
                if not ins.start_tensor_calc:
                    continue
                si = getattr(ins, "sync_info", None)
                if not si or not si.on_wait:
                    continue
                if r < lookback:
                    continue
                tgt = pe_positions[r - lookback]
                plan.append((ins, tgt, list(si.on_wait), list(si.on_update)))
            # back-to-front so earlier target indices stay valid
            for ins, tgt, waits, updates in sorted(plan, key=lambda p: -p[1]):
                nop = mybir.InstNoOp(
                    name=f"{ins.name}_hoistwait", ins=[], outs=[]
                )
                nop.engine = pe_engine
                nop.sync_info = mybir.SyncInfo(on_wait=waits, on_update=[])
                ins.sync_info = mybir.SyncInfo(on_wait=[], on_update=updates)
                insts.insert(tgt, nop)
    except AssertionError:
        pass
    return nc


def _split_multiwaits(nc):
    # This walrus build supports a single sync-wait slot per instruction;
    # hoist extra waits onto single-wait NoOps inserted just before the
    # offending instruction on the same engine (same semantics: the engine
    # stream blocks on each wait in order).
    n = 0
    for f in nc.m.functions:
        for blk in f.blocks:
            insts = blk.instructions  # live list
            i = 0
            while i < len(insts):
                inst = insts[i]
                si = getattr(inst, "sync_info", None)
                if si is not None and si.on_wait and len(si.on_wait) > 1:
                    waits = list(si.on_wait)
                    for j, w in enumerate(waits[:-1]):
                        nop = mybir.InstNoOp(
                            name=f"{inst.name}_splitwait{j}", ins=[], outs=[]
                        )
                        nop.engine = inst.engine
                        nop.sync_info = mybir.SyncInfo(on_wait=[w], on_update=[])
                        insts.insert(i, nop)
                        i += 1
                        n += 1
                    inst.sync_info = mybir.SyncInfo(
                        on_wait=[waits[-1]], on_update=list(si.on_update)
                    )
                i += 1
    return n


def build_nc():
    bf = mybir.dt.bfloat16
    f32 = mybir.dt.float32
    nc = bass.Bass()

    xt = nc.dram_tensor("xt", [NPASS, P, KTH, T], bf, kind="ExternalInput")
    wgu = nc.dram_tensor("wgu", [MTI, P, 2, KTH, P], bf, kind="ExternalInput")
    wd = nc.dram_tensor("wd", [MTH, P, KTI, P], bf, kind="ExternalInput")
    yt = nc.dram_tensor("yt", [NPASS, MTH, P, T], f32, kind="ExternalOutput")

    with tile.TileContext(nc) as tc:
        with (
            tc.tile_pool(name="xp", bufs=1) as xp,
            tc.tile_pool(name="hp", bufs=1) as hp,
            tc.tile_pool(name="wp", bufs=2) as wp,
            tc.tile_pool(name="sp", bufs=3) as sp,
            tc.tile_pool(name="pg", bufs=3, space="PSUM") as pg,
            tc.tile_pool(name="py", bufs=2, space="PSUM") as py,
        ):
            for ps in range(NPASS):
                xt_sb = xp.tile([P, KTH, T], bf, name="xt_sb", tag="xt_sb")
                # xt on the Scalar engine's DMA queue (parallel with the
                # weight stream on qSync), chunked so the first matmuls
                # start as soon as the first k-tiles land.
                for c in range(4):
                    k0, k1 = c * (KTH // 4), (c + 1) * (KTH // 4)
                    nc.scalar.dma_start(xt_sb[:, k0:k1], xt[ps, :, k0:k1])
                ht = hp.tile([P, MTI, T], bf, name="ht", tag="ht")
                for mt in range(MTI):
                    w = wp.tile([P, 2, KTH, P], bf, name="w_gu", tag="w")
                    if ps == 0 and mt == 0:
                        # chunk the very first weight tile so MM(kt=0) can
                        # start before the whole 2MB tile arrives
                        for c in range(4):
                            k0, k1 = c * (KTH // 4), (c + 1) * (KTH // 4)
                            nc.sync.dma_start(
                                w[:, :, k0:k1], wgu[mt, :, :, k0:k1]
                            )
                    else:
                        nc.sync.dma_start(w[:], wgu[mt])
                    g_ps = pg.tile([P, T], f32, name="g_ps", tag="g")
                    u_ps = pg.tile([P, T], f32, name="u_ps", tag="u")
                    for kt in range(KTH):
                        nc.tensor.matmul(
                            g_ps[:], w[:, 0, kt], xt_sb[:, kt],
                            start=(kt == 0), stop=(kt == KTH - 1),
                        )
                    for kt in range(KTH):
                        nc.tensor.matmul(
                            u_ps[:], w[:, 1, kt], xt_sb[:, kt],
                            start=(kt == 0), stop=(kt == KTH - 1),
                        )
                    sil = sp.tile([P, T], f32, name="sil", tag="sil")
                    nc.scalar.activation(
                        sil[:], g_ps[:], mybir.ActivationFunctionType.Silu
                    )
                    nc.vector.tensor_mul(ht[:, mt], sil[:], u_ps[:])
                for mh in range(MTH):
                    wdt = wp.tile([P, KTI, P], bf, name="w_d", tag="w")
                    nc.sync.dma_start(wdt[:], wd[mh])
                    y_ps = py.tile([P, T], f32, name="y_ps", tag="y")
                    for kt in range(KTI):
                        nc.tensor.matmul(
                            y_ps[:], wdt[:, kt], ht[:, kt],
                            start=(kt == 0), stop=(kt == KTI - 1),
                        )
                    y_sb = sp.tile([P, T], f32, name="y_sb", tag="ysb")
                    nc.vector.tensor_copy(y_sb[:], y_ps[:])
                    nc.sync.dma_start(yt[ps, mh], y_sb[:])

    _thin_mm_sem_updates(nc)
    # NOTE: _hoist_group_entry_waits is intentionally NOT called. Measured
    # 2026-08-11: hoisting the ~400 group-entry waits onto PE-queue NoOps
    # REGRESSED device time 3.590 -> 3.731 ms (+356 ns per NoOp) — an extra
    # instruction on the PE queue costs more than the ~216 ns boundary slot
    # it was meant to recover. Kept for documentation.
    _split_multiwaits(nc)
    return nc


def prep_inputs(x, W_gate, W_up, W_down):
    # lhsT layouts: element [mt, p, (g,) kt, m] = W[mt*128+m, kt*128+p]
    wg = W_gate.reshape(MTI, P, KTH, P).transpose(0, 3, 2, 1)
    wu = W_up.reshape(MTI, P, KTH, P).transpose(0, 3, 2, 1)
    wgu = np.stack([wg, wu], axis=2).astype(BF16)          # [mt, p, 2, kt, m]
    wd = W_down.reshape(MTH, P, KTI, P).transpose(0, 3, 2, 1).astype(BF16)
    # x: [core, pass, t, kt, p] -> per-core [pass, p, kt, t]
    xr = x.reshape(NCORES, NPASS, T, KTH, P)
    xts = [np.ascontiguousarray(xr[c].transpose(0, 3, 2, 1)).astype(BF16)
           for c in range(NCORES)]
    return xts, wgu, wd


_NC_CACHE = []


def get_nc():
    if not _NC_CACHE:
        _NC_CACHE.append(build_nc())
    return _NC_CACHE[0]


def make_in_maps(x, W_gate, W_up, W_down):
    xts, wgu, wd = prep_inputs(
        np.asarray(x, np.float32),
        np.asarray(W_gate, np.float32),
        np.asarray(W_up, np.float32),
        np.asarray(W_down, np.float32),
    )
    return [{"xt": xts[c], "wgu": wgu, "wd": wd} for c in range(NCORES)]


def unshard_output(res_by_name):
    yt = np.asarray(res_by_name["yt"]).reshape(NCORES, NPASS, MTH, P, T)
    out = np.empty((TOK, HID), np.float32)
    for c in range(NCORES):
        out[c * TPC:(c + 1) * TPC] = (
            yt[c].transpose(0, 3, 1, 2).reshape(TPC, HID)
        )
    return out


def kernel(x, W_gate, W_up, W_down):
    nc = get_nc()
    in_maps = make_in_maps(x, W_gate, W_up, W_down)
    res = run_bass_kernel_spmd(nc, in_maps, core_ids=list(range(NCORES)))
    return unshard_output(
        {"yt": np.stack([res.results[c]["yt"] for c in range(NCORES)]).reshape(
            NCORES * NPASS, MTH, P, T
        )}
    )

